# revision 70
# baseline (speedup 1.0000x reference)
"""MoE runtime-experts kernel for 8 Trainium2 NeuronCores.

Problem: y[t] = gelu(x[t] @ W1[e] + b1[e]) @ W2[e] + b2[e], e = indices[t].
T=8192 tokens, D=1024, H=4096, E=8 experts.

Strategy: expert-parallel. Host routes tokens by expert (argsort), core e
gets expert e's weights plus its tokens (transposed, zero-padded to a
common Tp so all 8 cores run one SPMD program). On device each core runs a
dense 2-layer MLP with fp32 PSUM accumulation:

  layer 1: hT[h, t] = gelu(sum_d W1[d, h] * xT[d, t] + b1[h])
           (lhsT = W1 k-tile [128d, 128h], rhs = xT [128d, 384t])
  layer 2: yT[d, t] = sum_h W2[h, d] * hT[h, t] + b2[d]
           (lhsT = W2 h-tile [128h, 128d], rhs = hT [128h, 384t])

Both layers keep the token axis in the free dimension, so no on-device
transpose is needed anywhere — and because tokens are always a free dim,
Tp needs no alignment: every core computes exactly max(counts) token
columns, split into balanced chunks of <=384 (one fp32 PSUM bank each).
Token-chunk DMAs are spread across the sync and gpsimd rings while the
scalar ring streams w1, so the PE starts ~13 us in and stays >=90% busy.
Host un-permutes yT shards into the full [T, 1, D] output.

KERNEL_MODE selects compute dtype: "fp8s" (default; both layers fp8e4m3
+ DoubleRow, with W1 sent as W1-0.5 and the exact rank-1 correction
c[t] = 0.5*sum_d x[d,t] computed on host in fp32 and added on-device by
the vector engine before gelu — this removes the common-mode error of
naive fp8 that fails the 2e-2 gate), "bf16", "fp8" (naive fp8, fails
the gate), "fp8l1" (layer 1 fp8, layer 2 bf16).
"""

import math
import os

import numpy as np
import ml_dtypes

T, D, H, E = 8192, 1024, 4096, 8
N_CORES = 8
KB_D = D // 128  # 8  k-tiles of the D contraction
HB = H // 128  # 32 h-tiles
DB = D // 128  # 8  d-tiles
BF16 = ml_dtypes.bfloat16
CS = 384  # token chunk (matmul moving-operand free dim)
SUP = 4 * CS  # tokens resident per pass (SBUF limit)
MM_N = 512  # PSUM bank free size (fp32)

MODE = os.environ.get("KERNEL_MODE", "fp8s")

_program_cache: dict[tuple, object] = {}
last_results = None  # BassKernelResults of the most recent kernel() call


def _chunk_sizes(Tp: int):
    """Balanced split of Tp token columns into chunks of at most CS."""
    nch = max(1, math.ceil(Tp / CS))
    base, rem = divmod(Tp, nch)
    return [base + (1 if i < rem else 0) for i in range(nch)]


def _build_program(Tp: int, mode: str):
    import concourse.tile as tile
    from concourse import bacc, mybir

    sizes = _chunk_sizes(Tp)
    nch = len(sizes)
    offs = [sum(sizes[:i]) for i in range(nch)]  # global token offsets

    f32 = mybir.dt.float32
    bf16 = mybir.dt.bfloat16
    fp8 = mybir.dt.float8e4
    l1_dt = fp8 if mode in ("fp8", "fp8l1", "fp8s") else bf16
    l2_dt = fp8 if mode in ("fp8", "fp8s") else bf16
    shifted = mode == "fp8s"
    l1_dr = l1_dt == fp8
    l2_dr = l2_dt == fp8
    dr = mybir.MatmulPerfMode.DoubleRow
    gelu = mybir.ActivationFunctionType.Gelu
    ident = mybir.ActivationFunctionType.Identity

    nc = bacc.Bacc(
        "TRN2", target_bir_lowering=False, debug=False, num_devices=N_CORES
    )

    # xq[c] is the SBUF image of token chunk c: [128, KB_D*CS], row-major
    # (kb, t) per partition, so the DMA is fully contiguous
    xq = nc.dram_tensor(
        "xq", [nch, 128, KB_D * CS], l1_dt, kind="ExternalInput"
    ).ap()
    # w1[h] is a [128, KB_D*128] block: col-chunk kb holds W1[kb*128+p, h*128+m]
    w1 = nc.dram_tensor(
        "w1", [HB, 128, KB_D * 128], l1_dt, kind="ExternalInput"
    ).ap()
    # w2[d] is a [128, HB*128] block: col-chunk hb holds W2[hb*128+p, d*128+m]
    w2 = nc.dram_tensor(
        "w2", [DB, 128, HB * 128], l2_dt, kind="ExternalInput"
    ).ap()
    # b1 and b2 concatenated so one DMA loads both (ring latency is
    # per-DMA, ~2 us, so early small transfers are batched)
    b12 = nc.dram_tensor(
        "b12", [128, HB + DB], f32, kind="ExternalInput"
    ).ap()
    # cq[c] = 0.5*colsum(x) for chunk c's tokens, replicated over the
    # 128 partitions (fp32; the rank-1 mean-shift correction for fp8s)
    cq = (
        nc.dram_tensor("cq", [nch, 128, CS], f32, kind="ExternalInput").ap()
        if shifted
        else None
    )
    yT = nc.dram_tensor("yT", [D, Tp], f32, kind="ExternalOutput").ap()

    def mm_group(ps, tsz, nk, lhs_of, rhs_of, use_dr):
        """Accumulate nk k-tiles into psum ps[:, :tsz]; DoubleRow fuses
        pairs of k-tiles per matmul via 3D APs."""
        if use_dr:
            for j in range(0, nk, 2):
                nc.tensor.matmul(
                    ps[:, :tsz],
                    lhs_of(j, 2),
                    rhs_of(j, 2),
                    start=(j == 0),
                    stop=(j == nk - 2),
                    perf_mode=dr,
                )
        else:
            for j in range(nk):
                nc.tensor.matmul(
                    ps[:, :tsz],
                    lhs_of(j, 1),
                    rhs_of(j, 1),
                    start=(j == 0),
                    stop=(j == nk - 1),
                )

    with tile.TileContext(nc) as tc:
        with (
            tc.tile_pool(name="const", bufs=1) as const_pool,
            tc.tile_pool(name="acts", bufs=1) as acts_pool,
            tc.tile_pool(name="xtp", bufs=3) as xt_pool,
            tc.tile_pool(name="w1p", bufs=4) as w1_pool,
            tc.tile_pool(name="w2p", bufs=2) as w2_pool,
            tc.tile_pool(name="outp", bufs=4) as out_pool,
            tc.tile_pool(name="psum", bufs=7, space="PSUM") as psum_pool,
            tc.tile_pool(name="warm", bufs=1, space="PSUM") as warm_pool,
        ):
            b12_sb = const_pool.tile([128, HB + DB], f32)
            b1_sb = b12_sb[:, :HB]
            b2_sb = b12_sb[:, HB:]

            # HAM warmup: dummy matmuls fill the PE while the first
            # DMAs land, so real matmuls start at 2.4 GHz instead of 1.2
            warm_sb = const_pool.tile([128, MM_N], l1_dt)
            nc.vector.memset(warm_sb[:], 0.0)
            warm_ps = warm_pool.tile([128, MM_N], f32, tag="warm")
            for _ in range(7):
                nc.tensor.matmul(
                    warm_ps[:, :MM_N],
                    warm_sb[:, :128],
                    warm_sb[:, :MM_N],
                    start=True,
                    stop=True,
                )


            for sup0 in range(0, nch, SUP // CS):

                cix = list(range(sup0, min(sup0 + SUP // CS, nch)))
                loffs = [offs[c] - offs[cix[0]] for c in cix]  # ht-local
                sup_len = sum(sizes[c] for c in cix)
                ht_sb = acts_pool.tile([128, HB, sup_len], l2_dt, tag="ht")

                # DMA rings serialize transfers end-to-end (~2-2.5 us
                # each, latency-dominated), so small early transfers are
                # batched and the queues ordered by first use:
                # sync=[xt0, w1 batches...], gpsimd=[xt1+xt2, w2...],
                # scalar=[w1[0], b12, cq0, cq1+cq2] (idle until gelu)
                ncx = len(cix)
                xts = []
                for ci, c in enumerate(cix):
                    xt_c = xt_pool.tile(
                        [128, KB_D, CS],
                        l1_dt,
                        tag=f"xt{ci}",
                        bufs=1,
                        name=f"xt{ci}",
                    )
                    (nc.sync if ci == 0 else nc.gpsimd).dma_start(
                        xt_c[:],
                        xq[c].rearrange("p (k m) -> p k m", k=KB_D),
                    )
                    xts.append(xt_c)
                if shifted and sup0 == 0:
                    cq_all = xt_pool.tile(
                        [128, nch, CS], f32, tag="cqa", bufs=1
                    )
                cqs = [cq_all[:, c] for c in cix] if shifted else []
                if shifted:
                    # chunk 0's c loads on the scalar ring in the ramp
                    # block below; the rest follow xt1/xt2 on gpsimd
                    for ci, c in enumerate(cix):
                        if ci > 0 or sup0 > 0:
                            nc.gpsimd.dma_start(cq_all[:, c], cq[c])

                # ---- layer 1: hT[h, c] ----
                def w1_load(h, eng):
                    w1t = w1_pool.tile(
                        [128, KB_D, 128],
                        l1_dt,
                        tag="w1t",
                        bufs=4,
                        name="w1t",
                    )
                    eng.dma_start(
                        w1t[:],
                        w1[h].rearrange("p (k m) -> p k m", k=KB_D),
                    )
                    return w1t

                def l1_group(h, w1t, ci):
                    c = cix[ci]
                    xt_c = xts[ci]
                    tsz = sizes[c]
                    lo = loffs[ci]
                    ps = psum_pool.tile([128, MM_N], f32, tag="ps")
                    mm_group(
                        ps,
                        tsz,
                        KB_D,
                        lambda j, w: w1t[:, j : j + w, :]
                        if w == 2
                        else w1t[:, j, :],
                        lambda j, w: xt_c[:, j : j + w, :tsz]
                        if w == 2
                        else xt_c[:, j, :tsz],
                        l1_dr,
                    )
                    if shifted:
                        # psum += c (per-token rank-1 mean correction)
                        nc.vector.scalar_tensor_tensor(
                            ps[:, :tsz],
                            ps[:, :tsz],
                            1.0,
                            cqs[ci][:, :tsz],
                            mybir.AluOpType.mult,
                            mybir.AluOpType.add,
                        )
                    nc.scalar.activation(
                        ht_sb[:, h, lo : lo + tsz],
                        ps[:, :tsz],
                        gelu,
                        bias=b1_sb[:, h : h + 1],
                    )

                w1_views = {}
                if sup0 == 0:
                    # scalar ring (idle until first gelu) carries the
                    # ramp constants: w1[0], b12, cq0; the w1 stream
                    # rides sync behind xt0
                    w1_views[0] = w1_load(0, nc.scalar)
                    nc.scalar.dma_start(b12_sb[:], b12[:])
                    if shifted:
                        nc.scalar.dma_start(cq_all[:, cix[0]], cq[cix[0]])
                    # preload the gelu/identity ACT tables behind the
                    # scalar DMA issues: otherwise the 1.28 us table
                    # load lands on the first real gelu's critical path
                    warm_act = const_pool.tile([128, 2], f32)
                    nc.scalar.activation(
                        warm_act[:, 0:1], warm_sb[:, 0:1], gelu
                    )
                    nc.scalar.activation(
                        warm_act[:, 1:2], warm_sb[:, 0:1], ident
                    )
                    h_w1_start = 1
                else:
                    w1_views[0] = w1_load(0, nc.sync)
                    h_w1_start = 1
                for h in range(h_w1_start, HB):
                    w1_views[h] = w1_load(h, nc.sync)

                if sup0 == 0:
                    # ramp: first two h-tiles chunk-major, matching DMA
                    # arrival order (xt0 first, then xt1, xt2), so the
                    # PE never waits on a late chunk during warm-up
                    for ci in range(ncx):
                        l1_group(0, w1_views[0], ci)
                        l1_group(1, w1_views[1], ci)
                    h_start = 2
                else:
                    h_start = 0
                for h in range(h_start, HB):
                    for ci in range(ncx):
                        l1_group(h, w1_views[h], ci)

                # ---- layer 2: yT[d, c] ----
                # w2 in pairs on the gpsimd (SWDGE) ring: parallel to
                # the w1 stream on sync, so d=0,1 prefetch early
                # w2 singles on the gpsimd (SWDGE) ring, double-buffered
                w2_views = {}
                for d0 in range(DB):
                    w2t = w2_pool.tile(
                        [128, HB, 128], l2_dt, tag="w2t", bufs=2
                    )
                    nc.gpsimd.dma_start(
                        w2t[:],
                        w2[d0].rearrange("p (k m) -> p k m", k=HB),
                    )
                    w2_views[d0] = w2t
                for d in range(DB):
                    w2t = w2_views[d]
                    for ci, c in enumerate(cix):
                        tsz = sizes[c]
                        lo = loffs[ci]
                        go = offs[c]
                        ps = psum_pool.tile([128, MM_N], f32, tag="ps")
                        mm_group(
                            ps,
                            tsz,
                            HB,
                            lambda j, w: w2t[:, j : j + w, :]
                            if w == 2
                            else w2t[:, j, :],
                            lambda j, w: ht_sb[:, j : j + w, lo : lo + tsz]
                            if w == 2
                            else ht_sb[:, j, lo : lo + tsz],
                            l2_dr,
                        )
                        ot = out_pool.tile([128, MM_N], f32, tag="ot")
                        if d == DB - 1 and tsz > 256:
                            # last d-tile: two pieces per chunk on
                            # different rings so issue+transfer overlap;
                            # the final chunk's pieces go on the two
                            # HWDGE rings (sync/scalar) — gpsimd's SWDGE
                            # adds ~1 us first-byte to the exposed tail
                            ring_pairs = [
                                (nc.scalar, nc.gpsimd),
                                (nc.gpsimd, nc.sync),
                                (nc.sync, nc.scalar),
                            ]
                            r0, r1 = ring_pairs[ci % 3]
                            cut = tsz - 128
                            pieces = [
                                (0, cut, r0),
                                (cut, tsz - cut, r1),
                            ]
                        else:
                            # bulk stores alternate sync/gpsimd (both
                            # idle during layer 2; scalar runs the ACTs)
                            eng = nc.sync if ci % 2 == 0 else nc.gpsimd
                            pieces = [(0, tsz, eng)]
                        for p0, psz, st_eng in pieces:
                            nc.scalar.activation(
                                ot[:, p0 : p0 + psz],
                                ps[:, p0 : p0 + psz],
                                ident,
                                bias=b2_sb[:, d : d + 1],
                            )
                            st_eng.dma_start(
                                yT[
                                    d * 128 : (d + 1) * 128,
                                    go + p0 : go + p0 + psz,
                                ],
                                ot[:, p0 : p0 + psz],
                            )

    nc.compile()
    return nc


def kernel(x, indices_s, weight1, weight2, bias1, bias2):
    from concourse import mybir
    from concourse.bass_utils import run_bass_kernel_spmd

    x = np.asarray(x, dtype=np.float32)
    idx = np.asarray(indices_s).astype(np.int64).ravel()
    w1_full = np.asarray(weight1, dtype=np.float32)
    w2_full = np.asarray(weight2, dtype=np.float32)
    b1_full = np.asarray(bias1, dtype=np.float32)
    b2_full = np.asarray(bias2, dtype=np.float32)

    order = np.argsort(idx, kind="stable")
    counts = np.bincount(idx, minlength=E)
    starts = np.concatenate([[0], np.cumsum(counts)])
    # tokens live in the free dim everywhere, so no alignment is needed:
    # every core computes exactly max(counts) token columns
    Tp = max(128, int(counts.max()))
    sizes = _chunk_sizes(Tp)
    nch = len(sizes)
    offs = np.concatenate([[0], np.cumsum(sizes)])

    mode = MODE
    key = (Tp, mode)
    nc = _program_cache.get(key)
    if nc is None:
        nc = _build_program(Tp, mode)
        _program_cache[key] = nc

    fp8_np = mybir.dt.np(mybir.dt.float8e4)
    l1_np = fp8_np if mode in ("fp8", "fp8l1", "fp8s") else BF16
    l2_np = fp8_np if mode in ("fp8", "fp8s") else BF16
    shifted = mode == "fp8s"
    w1_shift = np.float32(0.5) if shifted else np.float32(0.0)

    in_maps = []
    for e in range(E):
        toks = order[starts[e] : starts[e + 1]]
        # slot-aligned image: chunk c's tokens at columns [c*CS, c*CS+sizes[c])
        xTs = np.zeros((D, nch * CS), dtype=np.float32)
        for c in range(nch):
            lo, hi = offs[c], min(offs[c + 1], counts[e])
            if hi > lo:
                xTs[:, c * CS : c * CS + (hi - lo)] = x[toks[lo:hi]].T
        # [D, nch*CS] -> [nch, 128, KB_D*CS] chunk-major SBUF image
        xq = (
            np.ascontiguousarray(
                xTs.reshape(KB_D, 128, nch, CS).transpose(2, 1, 0, 3)
            )
            .reshape(nch, 128, KB_D * CS)
            .astype(l1_np)
        )
        w1r = (
            np.ascontiguousarray(
                (w1_full[e] - w1_shift)
                .reshape(KB_D, 128, HB, 128)
                .transpose(2, 1, 0, 3)
            )
            .reshape(HB, 128, KB_D * 128)
            .astype(l1_np)
        )
        w2r = (
            np.ascontiguousarray(
                w2_full[e].reshape(HB, 128, DB, 128).transpose(2, 1, 0, 3)
            )
            .reshape(DB, 128, HB * 128)
            .astype(l2_np)
        )
        b1d = b1_full[e].reshape(HB, 128).T
        b2d = b2_full[e].reshape(DB, 128).T
        b12d = np.ascontiguousarray(np.concatenate([b1d, b2d], axis=1))
        im = {"xq": xq, "w1": w1r, "w2": w2r, "b12": b12d}
        if shifted:
            # c[t] = 0.5 * sum_d x[t, d] in fp32, slot-aligned like xq,
            # replicated across the 128 partitions
            cvals = np.zeros((nch * CS,), dtype=np.float32)
            for c in range(nch):
                lo, hi = offs[c], min(offs[c + 1], counts[e])
                if hi > lo:
                    cvals[c * CS : c * CS + (hi - lo)] = (
                        0.5 * x[toks[lo:hi]].sum(axis=1)
                    )
            im["cq"] = np.ascontiguousarray(
                np.broadcast_to(
                    cvals.reshape(nch, 1, CS), (nch, 128, CS)
                )
            )
        in_maps.append(im)

    res = run_bass_kernel_spmd(
        nc,
        in_maps,
        list(range(N_CORES)),
        trace=os.environ.get("BASS_TRACE") == "1",
    )
    global last_results
    last_results = res

    out = np.empty((T, D), dtype=np.float32)
    for e in range(E):
        toks = order[starts[e] : starts[e + 1]]
        out[toks] = res.results[e]["yT"][:, : counts[e]].T
    if res.exec_time_ns is not None:
        print(f"HW exec time: {res.exec_time_ns} ns")
    return out[:, None, :]

